# revision 42
# baseline (speedup 1.0000x reference)
"""Trainium2 Bass kernel for CLIP-style symmetric contrastive loss.

Problem: image_features [8192, 1024] f32, text_features [8192, 1024] f32.
  loss = 0.5 * (CE(logits, diag) + CE(logits.T, diag)),
  logits = cosine_similarity(img, txt) / 0.07.

Distribution: shard image rows across 8 NeuronCores. Each core computes the
transposed slab S^T = txt_raw^T @ img_n  ([8192 txt cols, 1024 img rows]) in
64 chunks of 128 txt columns, using fp8 DoubleRow matmuls (txt raw fp8 as the
stationary operand, normalized img fp8 as the moving operand). The per-txt-col
normalization 1/(T*||txt_j||) folds into the exp activation's per-partition
scale, so the text matrix is never normalized on device. Activation accum_out
yields column sums for free; row sums accumulate on the Vector engine in bf16
and reduce via a ones-matmul.

Normalization factors and fp8/transposed layouts are staged on the host (the
O(N*D) part, ~0.1% of the FLOPs); the device does the O(N^2*D) compute: the
GEMM, 67M exps, global softmax reductions, and the diagonal dots. The colsum
AllReduce is split so the first half overlaps the GEMM's second half.

Math (C = 1/T upper-bounds every logit, so exp(S - C) <= 1 is stable):
  loss = C + (R + L - 2 * Draw') / (2N)
    R     = sum_i log sum_j exp(S_ij - C)
    L     = sum_j log sum_i exp(S_ij - C)
    Draw' = sum_i cos(img_i, txt_i) / T
"""
import threading
from contextlib import ExitStack

import ml_dtypes
import numpy as np

import concourse.bacc as bacc
import concourse.bass as bass
import concourse.bass_isa as bass_isa
import concourse.mybir as mybir
import concourse.tile as tile
from concourse.bass_utils import run_bass_kernel_spmd

F32 = mybir.dt.float32
BF16 = mybir.dt.bfloat16
FP8 = mybir.dt.float8e4
AF = mybir.ActivationFunctionType
ALU = mybir.AluOpType
DR = mybir.MatmulPerfMode.DoubleRow

N_CORES = 8
N = 8192
D = 1024
TEMPERATURE = 0.07


def build_nc(n=N, d=D, n_cores=N_CORES, no_collective=False, stop_after=None):
    """Build the SPMD Bass program (same program on every core)."""
    cexp = float(1.0 / TEMPERATURE)          # stabilizer: max possible logit
    rows = n // n_cores                      # image rows per core (1024)
    P = 128
    kt = d // P                              # contraction sub-tiles (8)
    n_ch = n // P                            # txt column chunks of 128 (64)

    nc = bacc.Bacc("TRN2", target_bir_lowering=False, debug=False,
                   num_devices=n_cores)
    imgn_t8 = nc.dram_tensor("imgn_t8", [d, rows], FP8,
                             kind="ExternalInput").ap()
    txt_t8 = nc.dram_tensor("txt_t8", [d, n], FP8, kind="ExternalInput").ap()
    to_t8 = nc.dram_tensor("to_t8", [d, rows], FP8, kind="ExternalInput").ap()
    rcp_s = nc.dram_tensor("rcp_s", [1, n], F32, kind="ExternalInput").ap()
    rcp_own = nc.dram_tensor("rcp_own", [1, rows], F32,
                             kind="ExternalInput").ap()
    ones = nc.dram_tensor("ones", [P, P], F32, kind="ExternalInput").ap()
    ones_b = nc.dram_tensor("ones_b", [P, P], BF16, kind="ExternalInput").ap()
    out = nc.dram_tensor("out", [1, 1], F32, kind="ExternalOutput").ap()

    with tile.TileContext(nc) as tc:
        _body(tc, imgn_t8, txt_t8, to_t8, rcp_s, rcp_own, ones, ones_b, out,
              n=n, d=d, rows=rows, P=P, kt=kt, n_ch=n_ch,
              cexp=cexp, n_cores=n_cores, no_collective=no_collective,
              stop_after=stop_after)
    nc.compile()
    return nc


def _body(tc, imgn_t8, txt_t8, to_t8, rcp_s, rcp_own, ones, ones_b, out, *,
          n, d, rows, P, kt, n_ch, cexp, n_cores, no_collective,
          stop_after=None):
    nc = tc.nc
    with ExitStack() as ctx:
        persist = ctx.enter_context(tc.tile_pool(name="persist", bufs=1))
        sqp = ctx.enter_context(tc.tile_pool(name="sqp", bufs=3))
        exp_p = ctx.enter_context(tc.tile_pool(name="exp_p", bufs=4))
        v1 = ctx.enter_context(tc.tile_pool(name="v1", bufs=4))
        ex_ps = ctx.enter_context(tc.tile_pool(name="ex_ps", bufs=4, space="PSUM"))
        dram = ctx.enter_context(tc.tile_pool(name="dram", bufs=1, space="DRAM"))

        txtT8 = persist.tile([P, kt, n], FP8, tag="txtT8")      # [d-part, k, j]
        imgT8 = persist.tile([P, kt, rows], FP8, tag="imgT8")   # [d-part, k, i]
        to_h8 = persist.tile([P, kt, rows], FP8, tag="to_h8")
        racc = persist.tile([P, rows], BF16, tag="racc")        # rowsum partial
        csacc = persist.tile([P, n_ch], F32, tag="csacc")       # colsum partial
        rcpT = persist.tile([P, n_ch], F32, tag="rcpT")         # 1/(T*|txt_j|)
        rcpo_row = persist.tile([1, rows], F32, tag="rcpo_row")
        ones_sb = persist.tile([P, P], F32, tag="ones")
        ones_bsb = persist.tile([P, P], BF16, tag="ones_bsb")
        cs_sb = persist.tile([P, n_ch], F32, tag="cs_sb")
        ln_cs = persist.tile([P, n_ch], BF16, tag="ln_cs")
        ebias = persist.tile([P, 1], F32, tag="ebias")
        sc = persist.tile([P, 8], F32, tag="sc")

        half = n // 2
        cbuf1 = dram.tile([1, half], F32, tag="cbuf1")
        cbuf1_out = dram.tile([1, half], F32, tag="cbuf1_out", addr_space="Shared")
        cbuf2 = dram.tile([1, half + 8], F32, tag="cbuf2")
        cbuf2_out = dram.tile([1, half + 8], F32, tag="cbuf2_out",
                              addr_space="Shared")

        nc.sync.dma_start(ones_sb[:], ones[:])
        nc.sync.dma_start(ones_bsb[:], ones_b[:])
        nc.gpsimd.memset(ebias[:], float(-cexp))

        # host-staged per-column exp scales, in chunk-partition layout
        nc.sync.dma_start(
            rcpT[:], rcp_s[0:1, :].rearrange("a (x p) -> (a p) x", p=P))
        nc.sync.dma_start(rcpo_row[0:1, :], rcp_own[0:1, :])

        # DMA priority: normalized img (small, gates the first matmul), text
        # first half (chunks 0-31), second half, then txt_own fp8 (only
        # needed for the diag dots around chunk 40)
        hn = n // 2
        for k in range(kt):
            nc.sync.dma_start(imgT8[:, k, :], imgn_t8[k * P:(k + 1) * P, :])
        for k in range(kt):
            nc.sync.dma_start(txtT8[:, k, 0:hn], txt_t8[k * P:(k + 1) * P, 0:hn])
        for k in range(kt):
            nc.sync.dma_start(txtT8[:, k, hn:n], txt_t8[k * P:(k + 1) * P, hn:n])
        for k in range(kt):
            nc.sync.dma_start(to_h8[:, k, :], to_t8[k * P:(k + 1) * P, :])

        # --- Phase C: main fp8 DoubleRow matmul + exp + reductions -----------
        HB = rows // 512                     # img halves per chunk (2)
        hch = n_ch // 2

        def diag_dots():
            # diag dots (for Draw'): d-major elementwise + ones-matmul
            dg_ps = ex_ps.tile([P, rows], F32, tag="ex")
            for k in range(kt):
                dsc = sqp.tile([P, rows], BF16, tag="sq")
                nc.vector.tensor_tensor(dsc[:], imgT8[:, k, :], to_h8[:, k, :],
                                        ALU.mult)
                for h in range(HB):
                    nc.tensor.matmul(dg_ps[:, h * 512:(h + 1) * 512],
                                     ones_bsb[:], dsc[:, h * 512:(h + 1) * 512],
                                     start=(k == 0), stop=(k == kt - 1))
            w1 = v1.tile([1, rows], F32, tag="w1")
            nc.vector.tensor_tensor(w1[0:1, :], dg_ps[0:1, :], rcpo_row[0:1, :],
                                    ALU.mult)
            nc.vector.tensor_reduce(sc[0:1, 1:2], w1[0:1, :],
                                    axis=mybir.AxisListType.X, op=ALU.add)

        for c in range(n_ch):
            if c == 40:
                diag_dots()
            mm = ex_ps.tile([P, rows], F32, tag="ex")
            for t in range(kt // 2):
                for h in range(HB):
                    nc.tensor.matmul(
                        mm[:, h * 512:(h + 1) * 512],
                        txtT8[:, 2 * t:2 * t + 2, c * P:(c + 1) * P],
                        imgT8[:, 2 * t:2 * t + 2, h * 512:(h + 1) * 512],
                        start=(t == 0), stop=(t == kt // 2 - 1),
                        perf_mode=DR)
            ex = exp_p.tile([P, rows], BF16, tag="exp")
            nc.scalar.activation(ex[:], mm[:], AF.Exp,
                                 bias=ebias[:, 0:1], scale=rcpT[:, c:c + 1],
                                 accum_out=csacc[:, c:c + 1])
            if c == 0:
                nc.vector.tensor_copy(racc[:], ex[:])
            else:
                nc.vector.tensor_tensor(racc[:], racc[:], ex[:], ALU.add)
            if c == hch - 1:
                # first half of colsums complete: overlap its AllReduce with
                # the second half of the GEMM
                nc.sync.dma_start(
                    cbuf1[0:1, :].rearrange("a (x p) -> (a p) x", p=P),
                    csacc[:, 0:hch])
                if no_collective:
                    nc.sync.dma_start(cbuf1_out[:], cbuf1[:])
                else:
                    nc.gpsimd.collective_compute(
                        "AllReduce", ALU.add,
                        replica_groups=[list(range(n_cores))],
                        ins=[cbuf1[:].opt()], outs=[cbuf1_out[:].opt()])
                # ln of global colsums half 1, off the sync queue (blocks on
                # the AllReduce; gpsimd queue is otherwise idle)
                nc.gpsimd.dma_start(
                    cs_sb[:, 0:hch],
                    cbuf1_out[0:1, :].rearrange("a (x p) -> (a p) x", p=P))
                lacc1 = v1.tile([P, 8], F32, tag="lacc")
                nc.scalar.activation(ln_cs[:, 0:hch], cs_sb[:, 0:hch], AF.Ln,
                                     accum_out=lacc1[:, 0:1])

        if stop_after == "C":
            nc.sync.dma_start(out[0:1, 0:1], csacc[0:1, 0:1])
            return

        # --- Phase D: local scalars ------------------------------------------
        # R_m = sum_i ln(rowsum_i): partition-reduce racc via ones-matmul
        for h in range(HB):
            rs = ex_ps.tile([P, rows], F32, tag="ex")
            nc.tensor.matmul(rs[0:1, 0:512], ones_bsb[:, 0:1],
                             racc[:, h * 512:(h + 1) * 512],
                             start=True, stop=True)
            lnr = v1.tile([P, 512], BF16, tag="lnr")
            nc.scalar.activation(lnr[0:1, :], rs[0:1, 0:512], AF.Ln,
                                 accum_out=sc[0:1, 2 + h:3 + h])
        nc.vector.tensor_tensor(sc[0:1, 0:1], sc[0:1, 2:3], sc[0:1, 3:4],
                                ALU.add)                         # R_m

        # ship partials: [colsums second half (4096), R_m, Draw'_m]
        nc.sync.dma_start(
            cbuf2[0:1, 0:half].rearrange("a (x p) -> (a p) x", p=P),
            csacc[:, hch:n_ch])
        nc.sync.dma_start(cbuf2[0:1, half:half + 2], sc[0:1, 0:2])

        if stop_after == "D":
            nc.sync.dma_start(out[0:1, 0:1], sc[0:1, 0:1])
            return

        # --- Phase E: second AllReduce + finish ------------------------------
        if no_collective:
            nc.sync.dma_start(cbuf2_out[:], cbuf2[:])
        else:
            nc.gpsimd.collective_compute(
                "AllReduce", ALU.add,
                replica_groups=[list(range(n_cores))],
                ins=[cbuf2[:].opt()], outs=[cbuf2_out[:].opt()])

        nc.gpsimd.dma_start(
            cs_sb[:, hch:n_ch],
            cbuf2_out[0:1, 0:half].rearrange("a (x p) -> (a p) x", p=P))
        lacc = v1.tile([P, 8], F32, tag="v1")
        nc.scalar.activation(ln_cs[:, hch:n_ch], cs_sb[:, hch:n_ch], AF.Ln,
                             accum_out=lacc[:, 1:2])
        nc.vector.tensor_tensor(lacc[:, 2:3], lacc1[:, 0:1], lacc[:, 1:2],
                                ALU.add)
        lps = ex_ps.tile([P, rows], F32, tag="ex")
        nc.tensor.matmul(lps[0:1, 0:1], ones_sb[:, 0:1], lacc[:, 2:3],
                         start=True, stop=True)                  # L
        rd = v1.tile([P, 8], F32, tag="v1")
        nc.gpsimd.dma_start(rd[0:1, 0:2], cbuf2_out[0:1, half:half + 2])

        # loss = cexp + (R + L - 2 * Draw') / (2N)
        fin = v1.tile([P, 8], F32, tag="v1")
        nc.vector.tensor_tensor(fin[0:1, 0:1], rd[0:1, 0:1], lps[0:1, 0:1],
                                ALU.add)                         # R + L
        nc.vector.tensor_scalar_mul(fin[0:1, 1:2], rd[0:1, 1:2], -2.0)
        nc.vector.tensor_tensor(fin[0:1, 2:3], fin[0:1, 0:1], fin[0:1, 1:2],
                                ALU.add)
        nc.scalar.activation(fin[0:1, 3:4], fin[0:1, 2:3], AF.Copy,
                             bias=float(cexp), scale=float(1.0 / (2 * n)))
        nc.sync.dma_start(out[0:1, 0:1], fin[0:1, 3:4])


def make_in_maps(image_features, text_features, n=N, d=D, n_cores=N_CORES):
    image_features = np.asarray(image_features, dtype=np.float32)
    text_features = np.asarray(text_features, dtype=np.float32)
    rows = n // n_cores
    txt_t8 = np.ascontiguousarray(text_features.T).astype(ml_dtypes.float8_e4m3)
    tn = np.sqrt((text_features.astype(np.float64) ** 2).sum(axis=1))
    rcp_s = (1.0 / (TEMPERATURE * np.maximum(tn, 1e-8))).astype(
        np.float32).reshape(1, n)
    inrm = np.sqrt((image_features.astype(np.float64) ** 2).sum(
        axis=1, keepdims=True))
    imgn = (image_features / np.maximum(inrm, 1e-8)).astype(np.float32)
    imgn_t8 = imgn.T.astype(ml_dtypes.float8_e4m3)
    ones = np.ones((128, 128), dtype=np.float32)
    ones_b = np.ones((128, 128), dtype=ml_dtypes.bfloat16)
    return [
        {
            "imgn_t8": np.ascontiguousarray(
                imgn_t8[:, m * rows:(m + 1) * rows]),
            "txt_t8": txt_t8,
            "to_t8": np.ascontiguousarray(
                txt_t8[:, m * rows:(m + 1) * rows]),
            "rcp_s": rcp_s,
            "rcp_own": np.ascontiguousarray(
                rcp_s[0:1, m * rows:(m + 1) * rows]),
            "ones": ones,
            "ones_b": ones_b,
        }
        for m in range(n_cores)
    ]


_CACHE = {}
_LOCK = threading.Lock()


def _get_nc():
    with _LOCK:
        if "nc" not in _CACHE:
            _CACHE["nc"] = build_nc()
        return _CACHE["nc"]


def kernel(image_features, text_features):
    image_features = np.asarray(image_features, dtype=np.float32)
    text_features = np.asarray(text_features, dtype=np.float32)
    assert image_features.shape == (N, D) and text_features.shape == (N, D)
    nc = _get_nc()
    in_maps = make_in_maps(image_features, text_features)
    res = run_bass_kernel_spmd(nc, in_maps, list(range(N_CORES)))
    val = np.float32(res.results[0]["out"][0, 0])
    return np.array(val, dtype=np.float32)
